# revision 34
# baseline (speedup 1.0000x reference)
"""Trainium2 Bass kernel for nn_AttentionBlock (B=8, T=2048, C=K=V=1024).

Strategy: data-parallel over batch B across 8 NeuronCores (1 batch element
per core, no collectives).

Algebraic reduction: with softmax over the QUERY axis (axis=1), per-key-row
additive constants cancel in the softmax. Writing
  S = (XWq+bq)(XWk+bk)^T = X (Wq Wk^T) X^T + [X Wq bk] 1^T + 1 [bq^T Wk^T X^T] + bq.bk
the last two terms are constant per S^T row (per t) and cancel; only
u = X (Wq bk) survives (added along the q/free axis). So the device computes
  G^T = M^T X^T          (M = 16 Wq Wk^T, host-precomputed, bf16)
  S^T[t,q] = sum_c XT[c,t] G^T[c,q] + u[q]
saving the entire K projection.

fp8 acceleration (perf_mode=DoubleRow: two K=128 contractions per pass, 2x
MACs/cycle at the same 512-col streaming time; both operands e4m3):
  - the whole S^T matmul: lhsT = X^T fp8 (host, unscaled), rhs = G^T fp8
    (written from PSUM, carries 16x via M); exp scale absorbs the 16x.
  - 6 of 8 c-tiles of the G^T matmul contraction (NBF=2 stay bf16).
  - the entire output matmul O = A^T-ish @ V via the delta trick.
    Softmax normalization: A[q,t] = E[q,t]/d[t], d = sum_q E. Decompose
    A = 1/Q + (A - 1/Q) with Q=2048: the rank-1 part contributes
    colmean(V) (computed EXACTLY on host in f64 from the bf16 operands),
    and the deviation dA = A - 1/Q has |dA| ~ 0.35/Q, so e4m3's 3.6% RMS
    relative error lands on the small deviation only:
      c[t] = d[t]/2048, r[t] = 1/c[t]
      dA8[t,q] = e4m3(E[t,q]*r[t] - 1)        (= (E-c)*2048/d, exact form)
      V8[t,v]  = e4m3(V[t,v])                 (unscaled, std ~0.58)
      O = dA8^T @ V8 * mask/2048 + colmean(V) * mask
    Plain fp8(E) fails the 2e-2 gate (sim 2.7e-2); the delta form with
    NBF=2 measures 1.937e-2 on HW (sim predicted 1.938e-2; gate 2e-2).
Engine balance in phase 2 (PE streams 3.54us/t-tile): exp+accum on ACT
(2.8us), the dA8 pass is split scalar/vector/gpsimd (Copy with per-
partition scale=r, bias=-1 == tensor_scalar mult/add), and the two u-adds
go one to vector one to gpsimd so no engine exceeds the PE per-tile time.

Per core phases (ordered for DMA ramp: p1b needs only m+xt):
  phase 1b: G^T = M^T@X^T  (qq-outer), PSUM -> fp8 SBUF pair-layout tiles
  phase 1a: V = X@Wv + bv, drained PSUM->e4m3 directly into pair tiles
  phase 2:  S^T per t-tile as two [P,1024] PSUM half-tiles; exp fused with
            accum_out row-sum; then c=d/2048, r=1/c, dA8 tiles
  phase 3:  O = dA8-pairs DR@ V8-pairs (16 passes/q-tile); drain adds the
            host rank-1 term (o0bc = 2048*colmean(V)) and multiplies by
            mask/2048 (host-prescaled); one DMA descriptor per output tile.

DMA: descriptor issue is ~0.6us each, serialized per engine; ramp-critical
descriptors round-robin over sync/scalar/gpsimd, the rest serialize on
sync in phase-1b consumption order. ~76 junk warmup matmuls keep the PE
HAM clock-gate open through the framework preamble + DMA ramp.

Host side: shard over batch, pre-transpose X to X^T (bf16 + unscaled fp8),
precompute M = 16 Wq Wk^T (f64 -> bf16 c-tiles 0..NBF-1, e4m3 pair-layout
rest), u_b = 16 * X_b (Wq bk), o0 = colmean(V_dev) in f64, and both the
raw mask and mask/2048 as per-token vectors.
"""

import sys

if "/opt/trn_rl_repo" not in sys.path:
    sys.path.insert(0, "/opt/trn_rl_repo")

import ml_dtypes
import numpy as np

import concourse.bass as bass
import concourse.tile as tile
from concourse import bacc, mybir
from concourse.bass import ts
from concourse.bass_utils import run_bass_kernel_spmd

B, T, C, K = 8, 2048, 1024, 1024
P = 128
CT = C // P  # 8 contraction tiles
KT_N = K // P  # 8 k tiles
TT = T // P  # 16 token tiles
TP = TT // 2  # 8 token-tile pairs (DoubleRow)
F32 = mybir.dt.float32
BF16 = mybir.dt.bfloat16
FP8 = mybir.dt.float8e4
EXP = mybir.ActivationFunctionType.Exp
COPY = mybir.ActivationFunctionType.Copy
DR = mybir.MatmulPerfMode.DoubleRow
MULT = mybir.AluOpType.mult
ADD = mybir.AluOpType.add
# X8 = X (unscaled), G8 = 16*G (via M scaled 16x); exp scale = 1/(32*16)
SCALE = 1.0 / 512.0
# phase-1b precision split: first NBF c-tiles of M bf16, rest fp8 DoubleRow
# (6/8 fp8 sims at 1.94e-2 end to end vs the 2e-2 gate)
NBF = 2
NDR = (CT - NBF) // 2

N_CORES = 8

# gpsimd takes a dA8 chunk (needs fp8 output support in its ucode);
# fall back to a scalar/vector-only split if that proves broken
GPSIMD_DA8 = True

# set by test.py to collect a profile
TRACE = False
LAST_RESULT = None


def _bcast(ap, parts=P):
    """Partition-broadcast a DRAM AP: prepend a [0, parts] partition dim."""
    return bass.AP(tensor=ap.tensor, offset=ap.offset, ap=[[0, parts]] + list(ap.ap))


DEBUG = False


def build():
    nc = bacc.Bacc(None, target_bir_lowering=False)
    dbg = {}
    if DEBUG:
        for nm, shape, dt in [
            ("dbg_dh0", [P, 1], F32), ("dbg_dh1", [P, 1], F32),
            ("dbg_c", [P, 1], F32), ("dbg_r", [P, 1], F32),
            ("dbg_at", [P, 16], BF16), ("dbg_a8", [P, 2048], FP8),
            ("dbg_v8", [P, 16], FP8), ("dbg_o0m", [P, 16], F32),
        ]:
            dbg[nm] = nc.declare_dram_parameter(nm, shape, dt, isOutput=True)

    xt_d = nc.declare_dram_parameter("xt", [C, T], BF16, isOutput=False)
    xt8_d = nc.declare_dram_parameter("xt8", [C, T], FP8, isOutput=False)
    m_d = nc.declare_dram_parameter("m", [KT_N, P, NBF, P], BF16, isOutput=False)
    m8_d = nc.declare_dram_parameter("m8", [KT_N, P, NDR, 2, P], FP8, isOutput=False)
    wv_d = nc.declare_dram_parameter("wv", [C, K], BF16, isOutput=False)
    bv_d = nc.declare_dram_parameter("bv", [K], F32, isOutput=False)
    u_d = nc.declare_dram_parameter("uvec", [T], F32, isOutput=False)
    o0_d = nc.declare_dram_parameter("o0v", [K], F32, isOutput=False)
    mask_d = nc.declare_dram_parameter("maskv", [T], F32, isOutput=False)
    mask2_d = nc.declare_dram_parameter("mask2v", [T], F32, isOutput=False)
    out_d = nc.declare_dram_parameter("out", [T, K], F32, isOutput=True)

    xt_r = xt_d[:, :].rearrange("(c p) t -> p c t", p=P)
    xt8_r = xt8_d[:, :].rearrange("(c p) t -> p c t", p=P)
    wv_r = wv_d[:, :].rearrange("(c p) k -> p c k", p=P)
    out_r = out_d[:, :].rearrange("(i p) v -> p i v", p=P)

    with tile.TileContext(nc) as tc:
        with (
            tc.tile_pool(name="const", bufs=1) as const,
            tc.tile_pool(name="qkv", bufs=1) as qkv,
            tc.tile_pool(name="p8", bufs=1) as p8,
            tc.tile_pool(name="psum", bufs=4, space="PSUM") as psum,
            tc.tile_pool(name="small", bufs=8) as small,
            tc.tile_pool(name="abuf", bufs=2) as abuf,
            tc.tile_pool(name="outp", bufs=3) as outp,
        ):
            # fp8 operands for phase 2 (persist through p2)
            xt8_sb = p8.tile([P, CT, T], FP8, tag="xt8")
            g8s = [p8.tile([P, 2, T], FP8, name=f"g8{j}", tag=f"g8{j}") for j in range(4)]
            # fp8 V pair tiles for phase 3 (written in p1a)
            v8s = [qkv.tile([P, 2, K], FP8, name=f"v8{j}", tag=f"v8{j}") for j in range(TP)]

            with tc.tile_pool(name="xtwv", bufs=1) as xtwv:
                xt_sb = xtwv.tile([P, CT, T], BF16, tag="xt")
                wv_sb = xtwv.tile([P, CT, K], BF16, tag="wv")
                m_ks = [xtwv.tile([P, NBF, P], BF16, name=f"mk{k}", tag=f"mk{k}") for k in range(KT_N)]
                m8_ks = [xtwv.tile([P, NDR, 2, P], FP8, name=f"m8k{k}", tag=f"m8k{k}") for k in range(KT_N)]

                # HAM warmup: keep the PE busy through the DMA ramp so the
                # clock gate opens before real matmuls begin (~9us).
                warm = const.tile([P, 64], BF16, tag="warm")
                nc.vector.memset(warm, 0.25)
                # load the Exp table now so phase 2's first tile isn't
                # stalled by ACT_TABLE_LOAD
                wact = const.tile([P, 1], BF16, tag="wact")
                nc.scalar.activation(out=wact, in_=warm[:, 0:1], func=EXP, scale=1.0)
                # 64 x ~56ns = 3.6us of junk matmuls: just over the HAM's
                # 4096-cycle (3.4us) busy window, and short enough that the
                # first real matmul isn't warmup-gated once DMA lands
                wps = psum.tile([P, 512], F32, tag="mm")
                for _w in range(64):
                    nc.tensor.matmul(
                        wps[0:64, 0:64], lhsT=warm, rhs=warm, start=True, stop=True
                    )

                # ---- input DMAs in strict priority order (descriptor issue
                # is ~0.6us each; per-queue stream ~22GB/s).
                bvbc = const.tile([P, K], F32, tag="bvbc")
                ubc = const.tile([P, T], F32, tag="ubc")
                o0bc = const.tile([P, K], F32, tag="o0bc")
                mask_sb = const.tile([P, TT], F32, tag="mask")
                mask2_sb = const.tile([P, TT], F32, tag="mask2")
                # ramp-critical + all m tiles round-robin over three engine
                # queues so phase 1b's k-groups never wait on descriptor
                # issue cadence; the remainder serializes on sync in
                # consumption order
                crit = [
                    (m_ks[0], m_d[0, :, :, :]),
                    (m8_ks[0], m8_d[0, :, :, :, :]),
                    (xt_sb[:, 0, ts(0, 512)], xt_r[:, 0, ts(0, 512)]),
                    (xt_sb[:, 1, ts(0, 512)], xt_r[:, 1, ts(0, 512)]),
                    (xt8_sb[:, NBF:8, ts(0, 512)], xt8_r[:, NBF:8, ts(0, 512)]),
                ]
                for k in range(1, KT_N):
                    crit.append((m_ks[k], m_d[k, :, :, :]))
                    crit.append((m8_ks[k], m8_d[k, :, :, :, :]))
                # the first seven descriptors gate phase 1b's k=0/k=1
                # groups: keep them on sync/scalar (gpsimd's DMA issue is
                # slower); the k>=2 m-pairs rotate through gpsimd as well
                crit_engs = [
                    nc.sync, nc.scalar, nc.sync, nc.scalar, nc.sync,
                    nc.scalar, nc.sync,
                ]
                rot = [nc.gpsimd, nc.sync, nc.scalar]
                crit_engs += [rot[i % 3] for i in range(len(crit) - 7)]
                for eng, (dst, src) in zip(crit_engs, crit):
                    eng.dma_start(out=dst, in_=src)
                rest = []
                for qq in range(1, 4):
                    rest.append((xt_sb[:, 0:NBF, ts(qq, 512)], xt_r[:, 0:NBF, ts(qq, 512)]))
                    rest.append((xt8_sb[:, NBF:8, ts(qq, 512)], xt8_r[:, NBF:8, ts(qq, 512)]))
                for ch in range(2):
                    rest.append((wv_sb[:, ts(ch, 4), :], wv_r[:, ts(ch, 4), :]))
                rest += [
                    (xt_sb[:, NBF:8, :], xt_r[:, NBF:8, :]),
                    (bvbc, _bcast(bv_d[:])),
                    (xt8_sb[:, 0:NBF, :], xt8_r[:, 0:NBF, :]),
                    (ubc, _bcast(u_d[:])),
                    (o0bc, _bcast(o0_d[:])),
                    (mask_sb, mask_d[:].rearrange("(i p) -> p i", p=P)),
                    (mask2_sb, mask2_d[:].rearrange("(i p) -> p i", p=P)),
                ]
                for dst, src in rest:
                    nc.sync.dma_start(out=dst, in_=src)

                # ---- phase 1b: G^T = M^T @ X^T -> fp8 pair-layout tiles.
                # qq-outer so the first chain needs only m_0 + xt qq-chunk 0.
                sc1b = nc.enter_named_scope("p1b_g", False)
                for qq in range(4):
                    for k in range(KT_N):
                        pg = psum.tile([P, 512], F32, tag="mm")
                        for c in range(NBF):
                            nc.tensor.matmul(
                                pg,
                                lhsT=m_ks[k][:, c, :],
                                rhs=xt_sb[:, c, ts(qq, 512)],
                                start=(c == 0),
                                stop=False,
                            )
                        for jp in range(NDR):
                            nc.tensor.matmul(
                                pg,
                                lhsT=m8_ks[k][:, jp, :, :],
                                rhs=xt8_sb[:, NBF + 2 * jp : NBF + 2 * jp + 2, ts(qq, 512)],
                                start=False,
                                stop=(jp == NDR - 1),
                                perf_mode=DR,
                            )
                        nc.vector.tensor_copy(g8s[k // 2][:, k % 2, ts(qq, 512)], pg)
                nc.leave_named_scope("p1b_g", sc1b[0], False)

                # ---- phase 1a: V = X @ Wv + bv -> e4m3 pair tiles directly
                sc1a = nc.enter_named_scope("p1a_v", False)
                for t in range(TT):
                    pv = psum.tile([P, K], F32, tag="mm")
                    for c in range(CT):
                        for h in range(2):
                            nc.tensor.matmul(
                                pv[:, ts(h, 512)],
                                lhsT=xt_sb[:, c, ts(t, P)],
                                rhs=wv_sb[:, c, ts(h, 512)],
                                start=(c == 0),
                                stop=(c == CT - 1),
                            )
                    nc.vector.tensor_add(v8s[t // 2][:, t % 2, :], pv, bvbc)
                nc.leave_named_scope("p1a_v", sc1a[0], False)

            # ---- phase 2: S^T via fp8 DoubleRow + u + exp; then the
            # normalized-deviation fp8 tiles dA8 = E*r - 1 (r = 2048/d).
            with tc.tile_pool(name="apool", bufs=1) as apool:
                sc2 = nc.enter_named_scope("p2_softmax", False)
                at8s = [
                    apool.tile([P, 2, T], FP8, name=f"a8{j}", tag=f"a8{j}")
                    for j in range(TP)
                ]
                # dA8 = E*r - 1 chunk split across engines, balanced under
                # the PE's 3.54us/t-tile streaming time. GpSimd has no PSUM
                # port so both u-adds stay on vector; gpsimd (SBUF-only)
                # takes the c-reduction and a dA8 chunk. GPSIMD_DA8 gates
                # fp8 output on gpsimd in case its ucode lacks the cast.
                if GPSIMD_DA8:
                    chunks = [(nc.scalar, 0, 512), (nc.vector, 512, 896), (nc.gpsimd, 896, 2048)]
                else:
                    chunks = [(nc.scalar, 0, 1024), (nc.vector, 1024, 2048)]
                for t in range(TT):
                    at = abuf.tile([P, T], BF16, tag="at")
                    dhs = []
                    for hh in range(2):
                        ps = psum.tile([P, K], F32, tag="mm")
                        for j in range(4):
                            for q2 in range(2):
                                nc.tensor.matmul(
                                    ps[:, ts(q2, 512)],
                                    lhsT=xt8_sb[:, 2 * j : 2 * j + 2, ts(t, P)],
                                    rhs=g8s[j][:, :, ts(2 * hh + q2, 512)],
                                    start=(j == 0),
                                    stop=(j == 3),
                                    perf_mode=DR,
                                )
                        half = slice(hh * 1024, (hh + 1) * 1024)
                        nc.vector.tensor_add(ps, ps, ubc[:, half])
                        dh = small.tile([P, 1], F32, tag="d")
                        nc.scalar.activation(
                            out=at[:, half],
                            in_=ps,
                            func=EXP,
                            scale=SCALE,
                            accum_out=dh,
                        )
                        dhs.append(dh)
                    # c = (dh0 + dh1)/2048 on gpsimd (SBUF-only), r = 1/c
                    c_t = small.tile([P, 1], F32, tag="c")
                    nc.gpsimd.tensor_scalar(
                        out=c_t,
                        in0=dhs[0],
                        scalar1=dhs[1][:, 0:1],
                        scalar2=1.0 / 2048.0,
                        op0=ADD,
                        op1=MULT,
                    )
                    r_t = small.tile([P, 1], F32, tag="r")
                    nc.vector.reciprocal(r_t, c_t)
                    dst = at8s[t // 2]
                    m2 = t % 2
                    if t == TT - 1:
                        # last tile's dA8 gates phase 3's final contraction
                        # pass; keep it off the slow gpsimd path (vector and
                        # scalar are about to go idle) to shrink the
                        # p2 -> p3 pipeline bubble
                        tchunks = [(nc.scalar, 0, 1024), (nc.vector, 1024, 2048)]
                    else:
                        tchunks = chunks
                    for eng, lo, hi in tchunks:
                        if eng is nc.scalar:
                            eng.activation(
                                out=dst[:, m2, lo:hi],
                                in_=at[:, lo:hi],
                                func=COPY,
                                scale=r_t[:, 0:1],
                                bias=-1.0,
                            )
                        else:
                            eng.tensor_scalar(
                                out=dst[:, m2, lo:hi],
                                in0=at[:, lo:hi],
                                scalar1=r_t[:, 0:1],
                                scalar2=-1.0,
                                op0=MULT,
                                op1=ADD,
                            )
                    if DEBUG and t == 0:
                        nc.sync.dma_start(out=dbg["dbg_dh0"][:, :], in_=dhs[0])
                        nc.sync.dma_start(out=dbg["dbg_dh1"][:, :], in_=dhs[1])
                        nc.sync.dma_start(out=dbg["dbg_c"][:, :], in_=c_t)
                        nc.sync.dma_start(out=dbg["dbg_r"][:, :], in_=r_t)
                        nc.sync.dma_start(out=dbg["dbg_at"][:, :], in_=at[:, 0:16])
                        nc.sync.dma_start(out=dbg["dbg_a8"][:, :], in_=dst[:, 0, :])
                        nc.sync.dma_start(out=dbg["dbg_v8"][:, :], in_=v8s[0][:, 0, 0:16])
                nc.leave_named_scope("p2_softmax", sc2[0], False)

                # ---- phase 3: O = dA8 DR@ V8 + o0; mask/2048 on the copy
                sc3 = nc.enter_named_scope("p3_out", False)
                for q in range(TT):
                    if q == TT - 1:
                        # separate half tiles so h0's drain can start while
                        # h1's matmuls still run (tile-granular tracking)
                        pos = [
                            psum.tile([P, 512], F32, name=f"po{h}", tag="mm")
                            for h in range(2)
                        ]
                        po = None
                    else:
                        po = psum.tile([P, K], F32, tag="mm")
                        pos = [po[:, ts(0, 512)], po[:, ts(1, 512)]]
                    for j in range(TP):
                        for h in range(2):
                            nc.tensor.matmul(
                                pos[h],
                                lhsT=at8s[j][:, :, ts(q, P)],
                                rhs=v8s[j][:, :, ts(h, 512)],
                                start=(j == 0),
                                stop=(j == TP - 1),
                                perf_mode=DR,
                            )
                    # out = po*(mask/2048) + o0bc*mask; the o0*mask product
                    # runs on the otherwise-idle scalar engine, then one
                    # fused vector op drains PSUM
                    o0m = outp.tile([P, K], F32, tag="o0m")
                    nc.scalar.activation(
                        out=o0m,
                        in_=o0bc,
                        func=COPY,
                        scale=mask_sb[:, q : q + 1],
                        bias=0.0,
                    )
                    if DEBUG and q == 0:
                        nc.sync.dma_start(out=dbg["dbg_o0m"][:, :], in_=o0m[:, 0:16])
                    ot = outp.tile([P, K], F32, tag="o")
                    if q == TT - 1:
                        # split the last tile's drain+DMA in halves so the
                        # tail overlaps (h0 drains while h1's matmuls finish)
                        for h in range(2):
                            hs = slice(h * 512, (h + 1) * 512)
                            nc.vector.scalar_tensor_tensor(
                                out=ot[:, hs],
                                in0=pos[h],
                                scalar=mask2_sb[:, q : q + 1],
                                in1=o0m[:, hs],
                                op0=MULT,
                                op1=ADD,
                            )
                            eng = nc.sync if h == 0 else nc.scalar
                            eng.dma_start(out=out_r[:, q, hs], in_=ot[:, hs])
                    else:
                        nc.vector.scalar_tensor_tensor(
                            out=ot,
                            in0=po,
                            scalar=mask2_sb[:, q : q + 1],
                            in1=o0m,
                            op0=MULT,
                            op1=ADD,
                        )
                        nc.sync.dma_start(out=out_r[:, q, :], in_=ot)
                nc.leave_named_scope("p3_out", sc3[0], False)

    nc.compile()
    return nc


def kernel(input, mask, Wq, bq, Wk, bk, Wv, bv):
    global LAST_RESULT
    input = np.asarray(input, dtype=np.float32)
    mask = np.asarray(mask, dtype=np.float32)
    bf = ml_dtypes.bfloat16
    e4 = ml_dtypes.float8_e4m3
    wq64 = np.asarray(Wq, dtype=np.float64)
    wk64 = np.asarray(Wk, dtype=np.float64)
    bk64 = np.asarray(bk, dtype=np.float64)
    # M scaled 16x so G lands in fp8's normal range; exp scale compensates
    m_f32 = (16.0 * (wq64 @ wk64.T)).astype(np.float32)
    # pre-slice M into [k_tile, p, c_tile, j]; c-tiles 0..NBF-1 stay bf16,
    # the rest go fp8 in DoubleRow pair layout [k, p, pair, 2, j]
    m_t = m_f32.reshape(CT, P, KT_N, P).transpose(2, 1, 0, 3)
    m_bf = np.ascontiguousarray(m_t[:, :, 0:NBF, :].astype(bf))
    m8_f = np.ascontiguousarray(
        m_t[:, :, NBF:8, :].reshape(KT_N, P, NDR, 2, P).astype(e4)
    )
    alpha = wq64 @ bk64  # [C]
    wv_bf = np.asarray(Wv, dtype=np.float32).astype(bf)
    wv_bf64 = wv_bf.astype(np.float64)
    bv64 = np.asarray(bv, dtype=np.float64)
    bv_f = np.ascontiguousarray(np.asarray(bv), dtype=np.float32)
    wv_c = np.ascontiguousarray(wv_bf)

    nc = build()
    in_maps = []
    for b in range(B):
        xt_f = input[b].T
        u_b = (input[b].astype(np.float64) @ alpha).astype(np.float32)
        # rank-1 softmax remainder: colmean(V_dev), V_dev from the
        # bf16-rounded operands so it matches the device projection exactly
        x_bf64 = input[b].astype(bf).astype(np.float64)
        o0 = x_bf64.mean(axis=0) @ wv_bf64 + bv64
        in_maps.append(
            {
                "xt": np.ascontiguousarray(xt_f.astype(bf)),
                "xt8": np.ascontiguousarray(xt_f.astype(e4)),
                "m": m_bf,
                "m8": m8_f,
                "wv": wv_c,
                "bv": bv_f,
                "uvec": np.ascontiguousarray(16.0 * u_b),
                "o0v": np.ascontiguousarray(o0.astype(np.float32)),
                "maskv": np.ascontiguousarray(mask[b, :, 0]),
                "mask2v": np.ascontiguousarray(mask[b, :, 0] / 2048.0),
            }
        )
    res = run_bass_kernel_spmd(nc, in_maps, list(range(N_CORES)), trace=TRACE)
    LAST_RESULT = res
    return np.stack([res.results[i]["out"] for i in range(N_CORES)], axis=0)


# revision 35
# speedup vs baseline: 1.0089x; 1.0089x over previous
"""Trainium2 Bass kernel for nn_AttentionBlock (B=8, T=2048, C=K=V=1024).

Strategy: data-parallel over batch B across 8 NeuronCores (1 batch element
per core, no collectives).

Algebraic reduction: with softmax over the QUERY axis (axis=1), per-key-row
additive constants cancel in the softmax. Writing
  S = (XWq+bq)(XWk+bk)^T = X (Wq Wk^T) X^T + [X Wq bk] 1^T + 1 [bq^T Wk^T X^T] + bq.bk
the last two terms are constant per S^T row (per t) and cancel; only
u = X (Wq bk) survives (added along the q/free axis). So the device computes
  G^T = M^T X^T          (M = 16 Wq Wk^T, host-precomputed, bf16)
  S^T[t,q] = sum_c XT[c,t] G^T[c,q] + u[q]
saving the entire K projection.

fp8 acceleration (perf_mode=DoubleRow: two K=128 contractions per pass, 2x
MACs/cycle at the same 512-col streaming time; both operands e4m3):
  - the whole S^T matmul: lhsT = X^T fp8 (host, unscaled), rhs = G^T fp8
    (written from PSUM, carries 16x via M); exp scale absorbs the 16x.
  - 6 of 8 c-tiles of the G^T matmul contraction (NBF=2 stay bf16).
  - the entire output matmul O = A^T-ish @ V via the delta trick.
    Softmax normalization: A[q,t] = E[q,t]/d[t], d = sum_q E. Decompose
    A = 1/Q + (A - 1/Q) with Q=2048: the rank-1 part contributes
    colmean(V) (computed EXACTLY on host in f64 from the bf16 operands),
    and the deviation dA = A - 1/Q has |dA| ~ 0.35/Q, so e4m3's 3.6% RMS
    relative error lands on the small deviation only:
      c[t] = d[t]/2048, r[t] = 1/c[t]
      dA8[t,q] = e4m3(E[t,q]*r[t] - 1)        (= (E-c)*2048/d, exact form)
      V8[t,v]  = e4m3(V[t,v])                 (unscaled, std ~0.58)
      O = dA8^T @ V8 * mask/2048 + colmean(V) * mask
    Plain fp8(E) fails the 2e-2 gate (sim 2.7e-2); the delta form with
    NBF=2 measures 1.937e-2 on HW (sim predicted 1.938e-2; gate 2e-2).
Engine balance in phase 2 (PE streams 3.54us/t-tile): exp+accum on ACT
(2.8us), the dA8 pass is split scalar/vector/gpsimd (Copy with per-
partition scale=r, bias=-1 == tensor_scalar mult/add), and the two u-adds
go one to vector one to gpsimd so no engine exceeds the PE per-tile time.

Per core phases (ordered for DMA ramp: p1b needs only m+xt):
  phase 1b: G^T = M^T@X^T  (qq-outer), PSUM -> fp8 SBUF pair-layout tiles
  phase 1a: V = X@Wv + bv, drained PSUM->e4m3 directly into pair tiles
  phase 2:  S^T per t-tile as two [P,1024] PSUM half-tiles; exp fused with
            accum_out row-sum; then c=d/2048, r=1/c, dA8 tiles
  phase 3:  O = dA8-pairs DR@ V8-pairs (16 passes/q-tile); drain adds the
            host rank-1 term (o0bc = 2048*colmean(V)) and multiplies by
            mask/2048 (host-prescaled); one DMA descriptor per output tile.

DMA: descriptor issue is ~0.6us each, serialized per engine; ramp-critical
descriptors round-robin over sync/scalar/gpsimd, the rest serialize on
sync in phase-1b consumption order. ~76 junk warmup matmuls keep the PE
HAM clock-gate open through the framework preamble + DMA ramp.

Host side: shard over batch, pre-transpose X to X^T (bf16 + unscaled fp8),
precompute M = 16 Wq Wk^T (f64 -> bf16 c-tiles 0..NBF-1, e4m3 pair-layout
rest), u_b = 16 * X_b (Wq bk), o0 = colmean(V_dev) in f64, and both the
raw mask and mask/2048 as per-token vectors.
"""

import sys

if "/opt/trn_rl_repo" not in sys.path:
    sys.path.insert(0, "/opt/trn_rl_repo")

import ml_dtypes
import numpy as np

import concourse.bass as bass
import concourse.tile as tile
from concourse import bacc, mybir
from concourse.bass import ts
from concourse.bass_utils import run_bass_kernel_spmd

B, T, C, K = 8, 2048, 1024, 1024
P = 128
CT = C // P  # 8 contraction tiles
KT_N = K // P  # 8 k tiles
TT = T // P  # 16 token tiles
TP = TT // 2  # 8 token-tile pairs (DoubleRow)
F32 = mybir.dt.float32
BF16 = mybir.dt.bfloat16
FP8 = mybir.dt.float8e4
EXP = mybir.ActivationFunctionType.Exp
COPY = mybir.ActivationFunctionType.Copy
DR = mybir.MatmulPerfMode.DoubleRow
MULT = mybir.AluOpType.mult
ADD = mybir.AluOpType.add
# X8 = X (unscaled), G8 = 16*G (via M scaled 16x); exp scale = 1/(32*16)
SCALE = 1.0 / 512.0
# phase-1b precision split: first NBF c-tiles of M bf16, rest fp8 DoubleRow
# (6/8 fp8 sims at 1.94e-2 end to end vs the 2e-2 gate)
NBF = 2
NDR = (CT - NBF) // 2

N_CORES = 8

# gpsimd takes a dA8 chunk (needs fp8 output support in its ucode);
# fall back to a scalar/vector-only split if that proves broken
GPSIMD_DA8 = True

# set by test.py to collect a profile
TRACE = False
LAST_RESULT = None


def _bcast(ap, parts=P):
    """Partition-broadcast a DRAM AP: prepend a [0, parts] partition dim."""
    return bass.AP(tensor=ap.tensor, offset=ap.offset, ap=[[0, parts]] + list(ap.ap))


DEBUG = False


def build():
    nc = bacc.Bacc(None, target_bir_lowering=False)
    dbg = {}
    if DEBUG:
        for nm, shape, dt in [
            ("dbg_dh0", [P, 1], F32), ("dbg_dh1", [P, 1], F32),
            ("dbg_c", [P, 1], F32), ("dbg_r", [P, 1], F32),
            ("dbg_at", [P, 16], BF16), ("dbg_a8", [P, 2048], FP8),
            ("dbg_v8", [P, 16], FP8), ("dbg_o0m", [P, 16], F32),
        ]:
            dbg[nm] = nc.declare_dram_parameter(nm, shape, dt, isOutput=True)

    xt_d = nc.declare_dram_parameter("xt", [C, T], BF16, isOutput=False)
    xt8_d = nc.declare_dram_parameter("xt8", [C, T], FP8, isOutput=False)
    m_d = nc.declare_dram_parameter("m", [KT_N, P, NBF, P], BF16, isOutput=False)
    m8_d = nc.declare_dram_parameter("m8", [KT_N, P, NDR, 2, P], FP8, isOutput=False)
    wv_d = nc.declare_dram_parameter("wv", [C, K], BF16, isOutput=False)
    bv_d = nc.declare_dram_parameter("bv", [K], F32, isOutput=False)
    u_d = nc.declare_dram_parameter("uvec", [T], F32, isOutput=False)
    o0_d = nc.declare_dram_parameter("o0v", [K], F32, isOutput=False)
    mask_d = nc.declare_dram_parameter("maskv", [T], F32, isOutput=False)
    mask2_d = nc.declare_dram_parameter("mask2v", [T], F32, isOutput=False)
    out_d = nc.declare_dram_parameter("out", [T, K], F32, isOutput=True)

    xt_r = xt_d[:, :].rearrange("(c p) t -> p c t", p=P)
    xt8_r = xt8_d[:, :].rearrange("(c p) t -> p c t", p=P)
    wv_r = wv_d[:, :].rearrange("(c p) k -> p c k", p=P)
    out_r = out_d[:, :].rearrange("(i p) v -> p i v", p=P)

    with tile.TileContext(nc) as tc:
        with (
            tc.tile_pool(name="const", bufs=1) as const,
            tc.tile_pool(name="qkv", bufs=1) as qkv,
            tc.tile_pool(name="p8", bufs=1) as p8,
            tc.tile_pool(name="psum", bufs=4, space="PSUM") as psum,
            tc.tile_pool(name="small", bufs=8) as small,
            tc.tile_pool(name="abuf", bufs=2) as abuf,
            tc.tile_pool(name="outp", bufs=3) as outp,
        ):
            # fp8 operands for phase 2 (persist through p2)
            xt8_sb = p8.tile([P, CT, T], FP8, tag="xt8")
            g8s = [p8.tile([P, 2, T], FP8, name=f"g8{j}", tag=f"g8{j}") for j in range(4)]
            # fp8 V pair tiles for phase 3 (written in p1a)
            v8s = [qkv.tile([P, 2, K], FP8, name=f"v8{j}", tag=f"v8{j}") for j in range(TP)]

            with tc.tile_pool(name="xtwv", bufs=1) as xtwv:
                xt_sb = xtwv.tile([P, CT, T], BF16, tag="xt")
                wv_sb = xtwv.tile([P, CT, K], BF16, tag="wv")
                m_ks = [xtwv.tile([P, NBF, P], BF16, name=f"mk{k}", tag=f"mk{k}") for k in range(KT_N)]
                m8_ks = [xtwv.tile([P, NDR, 2, P], FP8, name=f"m8k{k}", tag=f"m8k{k}") for k in range(KT_N)]

                # HAM warmup: keep the PE busy through the DMA ramp so the
                # clock gate opens before real matmuls begin (~9us).
                warm = const.tile([P, 64], BF16, tag="warm")
                nc.vector.memset(warm, 0.25)
                # load the Exp table now so phase 2's first tile isn't
                # stalled by ACT_TABLE_LOAD
                wact = const.tile([P, 1], BF16, tag="wact")
                nc.scalar.activation(out=wact, in_=warm[:, 0:1], func=EXP, scale=1.0)
                # 76 junk matmuls (~4.3us) keep the PE busy past the HAM's
                # 4096-cycle window AND through the DMA ramp; trimming to 64
                # measured slower (early phase-1b matmuls ran at the cold
                # 1.2GHz rate)
                wps = psum.tile([P, 512], F32, tag="mm")
                for _w in range(76):
                    nc.tensor.matmul(
                        wps[0:64, 0:64], lhsT=warm, rhs=warm, start=True, stop=True
                    )

                # ---- input DMAs in strict priority order (descriptor issue
                # is ~0.6us each; per-queue stream ~22GB/s).
                bvbc = const.tile([P, K], F32, tag="bvbc")
                ubc = const.tile([P, T], F32, tag="ubc")
                o0bc = const.tile([P, K], F32, tag="o0bc")
                mask_sb = const.tile([P, TT], F32, tag="mask")
                mask2_sb = const.tile([P, TT], F32, tag="mask2")
                # ramp-critical + all m tiles round-robin over three engine
                # queues so phase 1b's k-groups never wait on descriptor
                # issue cadence; the remainder serializes on sync in
                # consumption order
                crit = [
                    (m_ks[0], m_d[0, :, :, :]),
                    (m8_ks[0], m8_d[0, :, :, :, :]),
                    (xt_sb[:, 0, ts(0, 512)], xt_r[:, 0, ts(0, 512)]),
                    (xt_sb[:, 1, ts(0, 512)], xt_r[:, 1, ts(0, 512)]),
                    (xt8_sb[:, NBF:8, ts(0, 512)], xt8_r[:, NBF:8, ts(0, 512)]),
                ]
                for k in range(1, KT_N):
                    crit.append((m_ks[k], m_d[k, :, :, :]))
                    crit.append((m8_ks[k], m8_d[k, :, :, :, :]))
                # the first seven descriptors gate phase 1b's k=0/k=1
                # groups: keep them on sync/scalar (gpsimd's DMA issue is
                # slower); the k>=2 m-pairs rotate through gpsimd as well
                crit_engs = [
                    nc.sync, nc.scalar, nc.sync, nc.scalar, nc.sync,
                    nc.scalar, nc.sync,
                ]
                rot = [nc.gpsimd, nc.sync, nc.scalar]
                crit_engs += [rot[i % 3] for i in range(len(crit) - 7)]
                for eng, (dst, src) in zip(crit_engs, crit):
                    eng.dma_start(out=dst, in_=src)
                rest = []
                for qq in range(1, 4):
                    rest.append((xt_sb[:, 0:NBF, ts(qq, 512)], xt_r[:, 0:NBF, ts(qq, 512)]))
                    rest.append((xt8_sb[:, NBF:8, ts(qq, 512)], xt8_r[:, NBF:8, ts(qq, 512)]))
                for ch in range(2):
                    rest.append((wv_sb[:, ts(ch, 4), :], wv_r[:, ts(ch, 4), :]))
                rest += [
                    (xt_sb[:, NBF:8, :], xt_r[:, NBF:8, :]),
                    (bvbc, _bcast(bv_d[:])),
                    (xt8_sb[:, 0:NBF, :], xt8_r[:, 0:NBF, :]),
                    (ubc, _bcast(u_d[:])),
                    (o0bc, _bcast(o0_d[:])),
                    (mask_sb, mask_d[:].rearrange("(i p) -> p i", p=P)),
                    (mask2_sb, mask2_d[:].rearrange("(i p) -> p i", p=P)),
                ]
                for dst, src in rest:
                    nc.sync.dma_start(out=dst, in_=src)

                # ---- phase 1b: G^T = M^T @ X^T -> fp8 pair-layout tiles.
                # qq-outer so the first chain needs only m_0 + xt qq-chunk 0.
                sc1b = nc.enter_named_scope("p1b_g", False)
                for qq in range(4):
                    for k in range(KT_N):
                        pg = psum.tile([P, 512], F32, tag="mm")
                        for c in range(NBF):
                            nc.tensor.matmul(
                                pg,
                                lhsT=m_ks[k][:, c, :],
                                rhs=xt_sb[:, c, ts(qq, 512)],
                                start=(c == 0),
                                stop=False,
                            )
                        for jp in range(NDR):
                            nc.tensor.matmul(
                                pg,
                                lhsT=m8_ks[k][:, jp, :, :],
                                rhs=xt8_sb[:, NBF + 2 * jp : NBF + 2 * jp + 2, ts(qq, 512)],
                                start=False,
                                stop=(jp == NDR - 1),
                                perf_mode=DR,
                            )
                        nc.vector.tensor_copy(g8s[k // 2][:, k % 2, ts(qq, 512)], pg)
                nc.leave_named_scope("p1b_g", sc1b[0], False)

                # ---- phase 1a: V = X @ Wv + bv -> e4m3 pair tiles directly
                sc1a = nc.enter_named_scope("p1a_v", False)
                for t in range(TT):
                    pv = psum.tile([P, K], F32, tag="mm")
                    for c in range(CT):
                        for h in range(2):
                            nc.tensor.matmul(
                                pv[:, ts(h, 512)],
                                lhsT=xt_sb[:, c, ts(t, P)],
                                rhs=wv_sb[:, c, ts(h, 512)],
                                start=(c == 0),
                                stop=(c == CT - 1),
                            )
                    nc.vector.tensor_add(v8s[t // 2][:, t % 2, :], pv, bvbc)
                nc.leave_named_scope("p1a_v", sc1a[0], False)

            # ---- phase 2: S^T via fp8 DoubleRow + u + exp; then the
            # normalized-deviation fp8 tiles dA8 = E*r - 1 (r = 2048/d).
            with tc.tile_pool(name="apool", bufs=1) as apool:
                sc2 = nc.enter_named_scope("p2_softmax", False)
                at8s = [
                    apool.tile([P, 2, T], FP8, name=f"a8{j}", tag=f"a8{j}")
                    for j in range(TP)
                ]
                # dA8 = E*r - 1 chunk split across engines, balanced under
                # the PE's 3.54us/t-tile streaming time. GpSimd has no PSUM
                # port so both u-adds stay on vector; gpsimd (SBUF-only)
                # takes the c-reduction and a dA8 chunk. GPSIMD_DA8 gates
                # fp8 output on gpsimd in case its ucode lacks the cast.
                if GPSIMD_DA8:
                    chunks = [(nc.scalar, 0, 512), (nc.vector, 512, 896), (nc.gpsimd, 896, 2048)]
                else:
                    chunks = [(nc.scalar, 0, 1024), (nc.vector, 1024, 2048)]
                for t in range(TT):
                    at = abuf.tile([P, T], BF16, tag="at")
                    dhs = []
                    for hh in range(2):
                        ps = psum.tile([P, K], F32, tag="mm")
                        for j in range(4):
                            for q2 in range(2):
                                nc.tensor.matmul(
                                    ps[:, ts(q2, 512)],
                                    lhsT=xt8_sb[:, 2 * j : 2 * j + 2, ts(t, P)],
                                    rhs=g8s[j][:, :, ts(2 * hh + q2, 512)],
                                    start=(j == 0),
                                    stop=(j == 3),
                                    perf_mode=DR,
                                )
                        half = slice(hh * 1024, (hh + 1) * 1024)
                        nc.vector.tensor_add(ps, ps, ubc[:, half])
                        dh = small.tile([P, 1], F32, tag="d")
                        nc.scalar.activation(
                            out=at[:, half],
                            in_=ps,
                            func=EXP,
                            scale=SCALE,
                            accum_out=dh,
                        )
                        dhs.append(dh)
                    # c = (dh0 + dh1)/2048 on gpsimd (SBUF-only), r = 1/c
                    c_t = small.tile([P, 1], F32, tag="c")
                    nc.gpsimd.tensor_scalar(
                        out=c_t,
                        in0=dhs[0],
                        scalar1=dhs[1][:, 0:1],
                        scalar2=1.0 / 2048.0,
                        op0=ADD,
                        op1=MULT,
                    )
                    r_t = small.tile([P, 1], F32, tag="r")
                    nc.vector.reciprocal(r_t, c_t)
                    dst = at8s[t // 2]
                    m2 = t % 2
                    if t == TT - 1:
                        # last tile's dA8 gates phase 3's final contraction
                        # pass; keep it off the slow gpsimd path (vector and
                        # scalar are about to go idle) to shrink the
                        # p2 -> p3 pipeline bubble
                        tchunks = [(nc.scalar, 0, 1024), (nc.vector, 1024, 2048)]
                    else:
                        tchunks = chunks
                    for eng, lo, hi in tchunks:
                        if eng is nc.scalar:
                            eng.activation(
                                out=dst[:, m2, lo:hi],
                                in_=at[:, lo:hi],
                                func=COPY,
                                scale=r_t[:, 0:1],
                                bias=-1.0,
                            )
                        else:
                            eng.tensor_scalar(
                                out=dst[:, m2, lo:hi],
                                in0=at[:, lo:hi],
                                scalar1=r_t[:, 0:1],
                                scalar2=-1.0,
                                op0=MULT,
                                op1=ADD,
                            )
                    if DEBUG and t == 0:
                        nc.sync.dma_start(out=dbg["dbg_dh0"][:, :], in_=dhs[0])
                        nc.sync.dma_start(out=dbg["dbg_dh1"][:, :], in_=dhs[1])
                        nc.sync.dma_start(out=dbg["dbg_c"][:, :], in_=c_t)
                        nc.sync.dma_start(out=dbg["dbg_r"][:, :], in_=r_t)
                        nc.sync.dma_start(out=dbg["dbg_at"][:, :], in_=at[:, 0:16])
                        nc.sync.dma_start(out=dbg["dbg_a8"][:, :], in_=dst[:, 0, :])
                        nc.sync.dma_start(out=dbg["dbg_v8"][:, :], in_=v8s[0][:, 0, 0:16])
                nc.leave_named_scope("p2_softmax", sc2[0], False)

                # ---- phase 3: O = dA8 DR@ V8 + o0; mask/2048 on the copy
                sc3 = nc.enter_named_scope("p3_out", False)
                for q in range(TT):
                    if q == TT - 1:
                        # separate half tiles so h0's drain can start while
                        # h1's matmuls still run (tile-granular tracking)
                        pos = [
                            psum.tile([P, 512], F32, name=f"po{h}", tag="mm")
                            for h in range(2)
                        ]
                        po = None
                    else:
                        po = psum.tile([P, K], F32, tag="mm")
                        pos = [po[:, ts(0, 512)], po[:, ts(1, 512)]]
                    for j in range(TP):
                        for h in range(2):
                            nc.tensor.matmul(
                                pos[h],
                                lhsT=at8s[j][:, :, ts(q, P)],
                                rhs=v8s[j][:, :, ts(h, 512)],
                                start=(j == 0),
                                stop=(j == TP - 1),
                                perf_mode=DR,
                            )
                    # out = po*(mask/2048) + o0bc*mask; the o0*mask product
                    # runs on the otherwise-idle scalar engine, then one
                    # fused vector op drains PSUM
                    o0m = outp.tile([P, K], F32, tag="o0m")
                    nc.scalar.activation(
                        out=o0m,
                        in_=o0bc,
                        func=COPY,
                        scale=mask_sb[:, q : q + 1],
                        bias=0.0,
                    )
                    if DEBUG and q == 0:
                        nc.sync.dma_start(out=dbg["dbg_o0m"][:, :], in_=o0m[:, 0:16])
                    ot = outp.tile([P, K], F32, tag="o")
                    if q == TT - 1:
                        # split the last tile's drain+DMA in halves so the
                        # tail overlaps (h0 drains while h1's matmuls finish)
                        for h in range(2):
                            hs = slice(h * 512, (h + 1) * 512)
                            nc.vector.scalar_tensor_tensor(
                                out=ot[:, hs],
                                in0=pos[h],
                                scalar=mask2_sb[:, q : q + 1],
                                in1=o0m[:, hs],
                                op0=MULT,
                                op1=ADD,
                            )
                            eng = nc.sync if h == 0 else nc.scalar
                            eng.dma_start(out=out_r[:, q, hs], in_=ot[:, hs])
                    else:
                        nc.vector.scalar_tensor_tensor(
                            out=ot,
                            in0=po,
                            scalar=mask2_sb[:, q : q + 1],
                            in1=o0m,
                            op0=MULT,
                            op1=ADD,
                        )
                        nc.sync.dma_start(out=out_r[:, q, :], in_=ot)
                nc.leave_named_scope("p3_out", sc3[0], False)

    nc.compile()
    return nc


def kernel(input, mask, Wq, bq, Wk, bk, Wv, bv):
    global LAST_RESULT
    input = np.asarray(input, dtype=np.float32)
    mask = np.asarray(mask, dtype=np.float32)
    bf = ml_dtypes.bfloat16
    e4 = ml_dtypes.float8_e4m3
    wq64 = np.asarray(Wq, dtype=np.float64)
    wk64 = np.asarray(Wk, dtype=np.float64)
    bk64 = np.asarray(bk, dtype=np.float64)
    # M scaled 16x so G lands in fp8's normal range; exp scale compensates
    m_f32 = (16.0 * (wq64 @ wk64.T)).astype(np.float32)
    # pre-slice M into [k_tile, p, c_tile, j]; c-tiles 0..NBF-1 stay bf16,
    # the rest go fp8 in DoubleRow pair layout [k, p, pair, 2, j]
    m_t = m_f32.reshape(CT, P, KT_N, P).transpose(2, 1, 0, 3)
    m_bf = np.ascontiguousarray(m_t[:, :, 0:NBF, :].astype(bf))
    m8_f = np.ascontiguousarray(
        m_t[:, :, NBF:8, :].reshape(KT_N, P, NDR, 2, P).astype(e4)
    )
    alpha = wq64 @ bk64  # [C]
    wv_bf = np.asarray(Wv, dtype=np.float32).astype(bf)
    wv_bf64 = wv_bf.astype(np.float64)
    bv64 = np.asarray(bv, dtype=np.float64)
    bv_f = np.ascontiguousarray(np.asarray(bv), dtype=np.float32)
    wv_c = np.ascontiguousarray(wv_bf)

    nc = build()
    in_maps = []
    for b in range(B):
        xt_f = input[b].T
        u_b = (input[b].astype(np.float64) @ alpha).astype(np.float32)
        # rank-1 softmax remainder: colmean(V_dev), V_dev from the
        # bf16-rounded operands so it matches the device projection exactly
        x_bf64 = input[b].astype(bf).astype(np.float64)
        o0 = x_bf64.mean(axis=0) @ wv_bf64 + bv64
        in_maps.append(
            {
                "xt": np.ascontiguousarray(xt_f.astype(bf)),
                "xt8": np.ascontiguousarray(xt_f.astype(e4)),
                "m": m_bf,
                "m8": m8_f,
                "wv": wv_c,
                "bv": bv_f,
                "uvec": np.ascontiguousarray(16.0 * u_b),
                "o0v": np.ascontiguousarray(o0.astype(np.float32)),
                "maskv": np.ascontiguousarray(mask[b, :, 0]),
                "mask2v": np.ascontiguousarray(mask[b, :, 0] / 2048.0),
            }
        )
    res = run_bass_kernel_spmd(nc, in_maps, list(range(N_CORES)), trace=TRACE)
    LAST_RESULT = res
    return np.stack([res.results[i]["out"] for i in range(N_CORES)], axis=0)
